# revision 1
# baseline (speedup 1.0000x reference)
"""Causal self-attention Trainium2 kernel (8 NeuronCores).

Sharding: data-parallel over batch (4) x tensor-parallel over heads (2).
Core c handles batch b = c//2 and head group g = c%2 (8 of 16 heads,
feature slice [512*g, 512*(g+1))).

Per-core algorithm (T=2048, D=1024, local F=512, DK=64):
  qT/kT = Wl_aug.T @ xT_aug        [512, 2048]  (feature-major; bias via aug row)
  v     = xT_aug.T @ Wvl_aug       [2048, 512]  (token-major; bias via aug row)
  per head h, per query slab (512 cols):
    scoresT tile [tk=128, tq<=512] = kT_h_tile.T @ qT_h_cols   (K=64)
    probT = exp(scoresT/8)  (no max subtraction: |scores| <~ 10)
    yT[65, 512] += [v_h | 1].T @ probT    (row 64 = softmax denominator)
    yT_norm = yT[0:64] * bcast(1/denom)
  outT_partial [1024, 2048] = Wol.T @ yT (+ bo/2)
  ReduceScatter(add) over the core pair -> out shard [512, 2048]

Host: shards/transposes inputs, concatenates shards, transposes back.
All matmuls run as float32r (TRN2 fast fp32, ~2e-4 rel err).
"""
import sys, os
from contextlib import ExitStack

for _p in ("/opt/trn_rl_repo", "/root/.axon_site/_ro/trn_rl_repo"):
    if os.path.isdir(_p) and _p not in sys.path:
        sys.path.insert(0, _p)

import numpy as np

B, T, D, H = 4, 2048, 1024, 16
DK = D // H          # 64
N_CORES = 8
FL = D // 2          # 512 local features (8 heads)
HL = H // 2          # 8 local heads
SLAB = 512           # tq slab
NT = T // 128        # 16 token tiles
NS = T // SLAB       # 4 slabs
KC = D // 128        # 8 contraction chunks
NEG = -1.0e10

_CACHE = {}


def _build_nc(debug=False, repeat=1, parts="123", use_f32r=True, opts="MYP"):
    # parts flags: 1=phase1, 2=attention loop, 3=out-proj
    # sub-flags of 2 (auto-enabled if none given): F full scores/exp,
    # V full AV, D diag scores/exp, W diag AV, N normalize
    sub = set(parts) & set("FVDWN")
    if "2" in parts and not sub:
        sub = set("FVDWN")
    gF, gV, gD, gW, gN = ("F" in sub), ("V" in sub), ("D" in sub), ("W" in sub), ("N" in sub)
    import concourse.bass as bass
    import concourse.tile as tile
    from concourse import bacc, mybir

    F32 = mybir.dt.float32
    F32R = mybir.dt.float32r if use_f32r else mybir.dt.float32
    EXP = mybir.ActivationFunctionType.Exp
    IDENT = mybir.ActivationFunctionType.Identity
    ADD = mybir.AluOpType.add
    MULT = mybir.AluOpType.mult

    nc = bacc.Bacc("TRN2", target_bir_lowering=False, debug=False,
                   num_devices=N_CORES)

    xT = nc.dram_tensor("xT", [D + 1, T], F32R, kind="ExternalInput").ap()
    wq = nc.dram_tensor("wq", [D + 1, FL], F32R, kind="ExternalInput").ap()
    wk = nc.dram_tensor("wk", [D + 1, FL], F32R, kind="ExternalInput").ap()
    wv = nc.dram_tensor("wv", [D + 1, FL], F32R, kind="ExternalInput").ap()
    wo = nc.dram_tensor("wo", [FL, D], F32R, kind="ExternalInput").ap()
    bo2 = nc.dram_tensor("bo2", [128, D // 128], F32, kind="ExternalInput").ap()
    bqc = nc.dram_tensor("bqc", [128, FL // 128], F32, kind="ExternalInput").ap()
    bkc = nc.dram_tensor("bkc", [128, FL // 128], F32, kind="ExternalInput").ap()
    trimask = nc.dram_tensor("trimask", [128, 128], F32, kind="ExternalInput").ap()
    out_shard = nc.dram_tensor("out_shard", [FL, T], F32, kind="ExternalOutput").ap()
    if debug:
        qTd = nc.dram_tensor("qTd", [FL, T], F32, kind="ExternalOutput").ap()
        kTd = nc.dram_tensor("kTd", [FL, T], F32, kind="ExternalOutput").ap()
        vd = nc.dram_tensor("vd", [NT * 128, HL * (DK + 1)], F32, kind="ExternalOutput").ap()
        yTd = nc.dram_tensor("yTd", [FL, T], F32, kind="ExternalOutput").ap()
        outTd = nc.dram_tensor("outTd", [D, T], F32, kind="ExternalOutput").ap()

    with tile.TileContext(nc) as tc:
        with tc.tile_pool(name="const", bufs=1) as constp, \
             tc.tile_pool(name="psum", bufs=2, space="PSUM") as pp, \
             tc.tile_pool(name="dram", bufs=1, space="DRAM") as dram:

            # ---- constants ----
            m_sb = constp.tile([128, 128], F32, tag="m")
            nc.sync.dma_start(out=m_sb[:], in_=trimask[:])
            bo_sb = constp.tile([128, D // 128], F32, tag="bo")
            nc.sync.dma_start(out=bo_sb[:], in_=bo2[:])
            bq_sb = constp.tile([128, FL // 128], F32, tag="bqc")
            nc.sync.dma_start(out=bq_sb[:], in_=bqc[:])
            bk_sb = constp.tile([128, FL // 128], F32, tag="bkc")
            nc.sync.dma_start(out=bk_sb[:], in_=bkc[:])
            ones_f = constp.tile([1, 64], F32, tag="onesf")
            ones64 = constp.tile([1, 64], F32R, tag="ones")
            nc.vector.memset(ones_f[:], 1.0)
            nc.vector.tensor_copy(ones64[:], ones_f[:])
            ones128 = constp.tile([128, HL], F32, tag="ones128")
            nc.vector.memset(ones128[:], 1.0)

            outT_dram = dram.tile([D, T], F32)
            rs_out = dram.tile([FL, T], F32)

            for rep in range(repeat):
                R = f"r{rep}_"
                with ExitStack() as st:
                    persist = st.enter_context(tc.tile_pool(name=R + "persist", bufs=1))
                    if "Q" in opts:
                        qTbig = persist.tile([128, 4 * T], F32R, tag="qTbig", name=R + "qTbig")
                        kTbig = persist.tile([128, 4 * T], F32R, tag="kTbig", name=R + "kTbig")
                        qT = [qTbig[:, T * i:T * (i + 1)] for i in range(4)]
                        kT = [kTbig[:, T * i:T * (i + 1)] for i in range(4)]
                    else:
                        qT = [persist.tile([128, T], F32R, tag=f"qT{i}", name=R + f"qT{i}")[:]
                              for i in range(4)]
                        kT = [persist.tile([128, T], F32R, tag=f"kT{i}", name=R + f"kT{i}")[:]
                              for i in range(4)]
                    v_sb = [persist.tile([128, HL * (DK + 1)], F32R, tag=f"v{j}",
                                         name=R + f"v{j}") for j in range(NT)]
                    # ones column per head slice of v (col DK of each 65-block)
                    for j in range(NT):
                        ones_col = bass.AP(
                            tensor=v_sb[j].tensor, offset=v_sb[j].offset + DK,
                            ap=[list(v_sb[j].ap[0]), [DK + 1, HL]])
                        nc.vector.tensor_copy(ones_col, ones128[:])

                    # ================= phase 1: projections =================
                    with tc.tile_pool(name=R + "ph1", bufs=1) as ph1:
                        wq_sb = [ph1.tile([128, FL], F32R, tag=f"wq{kc}",
                                          name=R + f"wq_sb{kc}") for kc in range(KC)]
                        wk_sb = [ph1.tile([128, FL], F32R, tag=f"wk{kc}",
                                          name=R + f"wk_sb{kc}") for kc in range(KC)]
                        wv_sb = [ph1.tile([128, FL], F32R, tag=f"wv{kc}",
                                          name=R + f"wv_sb{kc}") for kc in range(KC)]
                        wqa = ph1.tile([1, FL], F32R, tag="wqa", name=R + "wqa")
                        wka = ph1.tile([1, FL], F32R, tag="wka", name=R + "wka")
                        wva = ph1.tile([1, FL], F32R, tag="wva", name=R + "wva")
                        for kc in range(KC):
                            nc.sync.dma_start(out=wq_sb[kc][:], in_=wq[128 * kc:128 * (kc + 1), :])
                            nc.sync.dma_start(out=wk_sb[kc][:], in_=wk[128 * kc:128 * (kc + 1), :])
                            nc.sync.dma_start(out=wv_sb[kc][:], in_=wv[128 * kc:128 * (kc + 1), :])
                        nc.sync.dma_start(out=wqa[:], in_=wq[D:D + 1, :])
                        nc.sync.dma_start(out=wka[:], in_=wk[D:D + 1, :])
                        nc.sync.dma_start(out=wva[:], in_=wv[D:D + 1, :])

                        with tc.tile_pool(name=R + "xa", bufs=12) as xap, \
                             tc.tile_pool(name=R + "xa1", bufs=2) as xap1:
                            for js in range(NS):
                                sl = slice(SLAB * js, SLAB * (js + 1))
                                xa = [xap.tile([128, SLAB], F32R, tag="xa",
                                               name=R + f"xa{js}_{kc}") for kc in range(KC)]
                                for kc in range(KC):
                                    nc.sync.dma_start(out=xa[kc][:],
                                                      in_=xT[128 * kc:128 * (kc + 1), sl])
                                xo = xap1.tile([1, SLAB], F32R, tag="xa1", name=R + f"xo{js}")
                                nc.sync.dma_start(out=xo[:], in_=xT[D:D + 1, sl])

                                # q and k: feature-major [f, t]
                                for wsb, wa, dst, big, bias, eng in (
                                        (wq_sb, wqa, qT, "qTbig", bq_sb, "act"),
                                        (wk_sb, wka, kT, "kTbig", bk_sb, "dve")):
                                    for half in range(2):  # fc pairs (0,1) and (2,3)
                                        ps = pp.tile([128, 1024], F32, tag="smm")
                                        for u in range(2):
                                            fc = 2 * half + u
                                            fsl = slice(128 * fc, 128 * (fc + 1))
                                            po = ps[:, 512 * u:512 * (u + 1)]
                                            for kc in range(KC):
                                                nc.tensor.matmul(po, wsb[kc][:, fsl], xa[kc][:],
                                                                 start=(kc == 0),
                                                                 stop=("G" in opts and kc == KC - 1))
                                            if "G" not in opts:
                                                nc.tensor.matmul(po, wa[:, fsl], xo[:],
                                                                 start=False, stop=True)
                                        if "Q" in opts:
                                            # one strided DVE op for both chunks
                                            fc0 = 2 * half
                                            t0 = dst[fc0].tensor
                                            dd = bass.AP(
                                                tensor=t0, offset=dst[fc0].offset + SLAB * js,
                                                ap=[list(dst[fc0].ap[0]), [T, 2], [1, SLAB]])
                                            s2 = ps[:].rearrange("p (a b) -> p a b", a=2)
                                            if "G" in opts:
                                                brep = bass.AP(
                                                    tensor=bias.tensor,
                                                    offset=bias.offset + fc0,
                                                    ap=[list(bias.ap[0]), [1, 2], [0, SLAB]])
                                                nc.vector.tensor_tensor(out=dd, in0=s2,
                                                                        in1=brep, op=ADD)
                                            else:
                                                nc.vector.tensor_copy(dd, s2)
                                        else:
                                            for u in range(2):
                                                fc = 2 * half + u
                                                src = ps[:, 512 * u:512 * (u + 1)]
                                                if "G" in opts:
                                                    if eng == "act":
                                                        nc.scalar.activation(
                                                            dst[fc][:, sl], src, IDENT,
                                                            bias=bias[:, fc:fc + 1], scale=1.0)
                                                    else:
                                                        nc.vector.tensor_scalar_add(
                                                            dst[fc][:, sl], src,
                                                            bias[:, fc:fc + 1])
                                                elif eng == "act":
                                                    nc.scalar.copy(dst[fc][:, sl], src)
                                                else:
                                                    nc.vector.tensor_copy(dst[fc][:, sl], src)

                                # v: token-major [t, f] with aug bias row
                                for half in range(2):
                                    ps = pp.tile([128, 1024], F32, tag="smm")
                                    for u in range(2):
                                        tt = 4 * js + 2 * half + u
                                        tsl = slice(128 * (2 * half + u),
                                                    128 * (2 * half + u + 1))
                                        po = ps[:, 512 * u:512 * (u + 1)]
                                        for kc in range(KC):
                                            nc.tensor.matmul(po, xa[kc][:, tsl], wv_sb[kc][:],
                                                             start=(kc == 0), stop=False)
                                        nc.tensor.matmul(po, xo[:, tsl], wva[:],
                                                         start=False, stop=True)
                                    for u in range(2):
                                        tt = 4 * js + 2 * half + u
                                        src3 = ps[:, 512 * u:512 * (u + 1)].rearrange(
                                            "p (h d) -> p h d", h=HL)
                                        dst3 = bass.AP(
                                            tensor=v_sb[tt].tensor, offset=v_sb[tt].offset,
                                            ap=[list(v_sb[tt].ap[0]), [DK + 1, HL], [1, DK]])
                                        nc.vector.tensor_copy(dst3, src3)

                    if debug and rep == repeat - 1:
                        for fc in range(4):
                            nc.sync.dma_start(out=qTd[128 * fc:128 * (fc + 1), :],
                                              in_=qT[fc][:].bitcast(F32))
                            nc.sync.dma_start(out=kTd[128 * fc:128 * (fc + 1), :],
                                              in_=kT[fc][:].bitcast(F32))
                        for j in range(NT):
                            nc.sync.dma_start(out=vd[128 * j:128 * (j + 1), :],
                                              in_=v_sb[j][:].bitcast(F32))

                    # ============ phase 2+3: attention + out-proj ============
                    p2 = st.enter_context(tc.tile_pool(name=R + "p2", bufs=1))
                    work = st.enter_context(tc.tile_pool(
                        name=R + "work", bufs=4 if "P" in opts else 3))
                    yT = [p2.tile([128, T], F32R, tag=f"yT{i}", name=R + f"yT{i}")
                          for i in range(4)]
                    wo_sb = [p2.tile([128, D], F32R, tag=f"wo{fc}", name=R + f"wo_sb{fc}")
                             for fc in range(4)]
                    for fc in range(4):
                        nc.sync.dma_start(out=wo_sb[fc][:], in_=wo[128 * fc:128 * (fc + 1), :])

                    for js in range(NS if "2" in parts else 0):
                        sl = slice(SLAB * js, SLAB * (js + 1))
                        for h in range(HL):
                            hp, off = h // 2, 64 * (h % 2)
                            hsl = slice(off, off + 64)
                            vsl = slice((DK + 1) * h, (DK + 1) * (h + 1))
                            qh = qT[hp][hsl, sl]
                            yp = pp.tile([65, 512], F32, tag="yacc")
                            n_full = 4 * js
                            # full (unmasked) tk tiles, in groups of 2
                            for gi in range(n_full // 2 if gF else 0):
                                ps = pp.tile([128, 1024], F32, tag="smm")
                                for u in range(2):
                                    j = 2 * gi + u
                                    nc.tensor.matmul(ps[:, 512 * u:512 * (u + 1)],
                                                     kT[hp][hsl, 128 * j:128 * (j + 1)], qh,
                                                     start=True, stop=True)
                                pr = work.tile([128, 1024], F32R, tag="prob")
                                nc.scalar.activation(pr[:], ps[:], EXP, scale=0.125)
                                for u in range(2 if gV else 0):
                                    j = 2 * gi + u
                                    nc.tensor.matmul(yp[:], v_sb[j][:, vsl],
                                                     pr[:, 512 * u:512 * (u + 1)],
                                                     start=(gi == 0 and u == 0), stop=False,
                                                     skip_group_check=True)
                            # diagonal region: tk tile 4js+jl covers tq cols
                            # [128*jl, 512); triangular mask on first 128 cols.
                            for grp, members in enumerate((((0, 0), (1, 512)),
                                                           ((2, 0), (3, 256))) if gD else ()):
                                wtot = (512 + 384, 256 + 128)[grp]
                                if grp == 0:
                                    pd = pp.tile([128, 1024], F32, tag="smm")
                                else:
                                    pd = pp.tile([128, 512], F32, tag="sdiag")
                                for jl, poff in members:
                                    j, col0 = 4 * js + jl, 128 * jl
                                    w = 512 - col0
                                    nc.tensor.matmul(pd[:, poff:poff + w],
                                                     kT[hp][hsl, 128 * j:128 * (j + 1)],
                                                     qh[:, col0:512],
                                                     start=True, stop=True)
                                if "M" in opts:
                                    # both diagonal 128-blocks in one DVE op
                                    stride = members[1][1]
                                    pdm = bass.AP(tensor=pd.tensor, offset=pd.offset,
                                                  ap=[list(pd.ap[0]), [stride, 2], [1, 128]])
                                    mr2 = bass.AP(tensor=m_sb.tensor, offset=m_sb.offset,
                                                  ap=[list(m_sb.ap[0]), [0, 2], [1, 128]])
                                    nc.vector.tensor_tensor(out=pdm, in0=pdm, in1=mr2, op=ADD)
                                else:
                                    for jl, poff in members:
                                        nc.vector.tensor_tensor(
                                            out=pd[:, poff:poff + 128],
                                            in0=pd[:, poff:poff + 128], in1=m_sb[:], op=ADD)
                                prd = work.tile([128, 1024], F32R, tag="prob")
                                nc.scalar.activation(prd[:, 0:wtot], pd[:, 0:wtot],
                                                     EXP, scale=0.125)
                                for jl, poff in (members if gW else ()):
                                    j, col0 = 4 * js + jl, 128 * jl
                                    w = 512 - col0
                                    nc.tensor.matmul(yp[:, col0:512], v_sb[j][:, vsl],
                                                     prd[:, poff:poff + w],
                                                     start=(js == 0 and jl == 0),
                                                     stop=(jl == 3),
                                                     skip_group_check=True)
                            # normalize: yT = yp[0:64] * bcast(1 / yp[64])
                            if not gN:
                                continue
                            rec = work.tile([1, 512], F32R, tag="rec", name=R + "rec")
                            with nc.allow_low_precision(reason="f32r is rounded fp32"):
                                nc.vector.reciprocal(rec[:], yp[64:65, :])
                            pb = pp.tile([128, 512], F32, tag="sdiag")
                            nc.tensor.matmul(pb[0:64, :], ones64[:], rec[:],
                                             start=True, stop=True)
                            yun = work.tile([64, 512], F32, tag="yun", name=R + "yun")
                            if "Y" in opts:
                                nc.scalar.copy(yun[:], yp[0:64, :])
                            else:
                                nc.vector.tensor_copy(yun[:], yp[0:64, :])
                            nc.vector.tensor_tensor(out=yT[hp][hsl, sl], in0=yun[:],
                                                    in1=pb[0:64, :], op=MULT)

                        # out-projection for this slab
                        if "3" in parts and "D" in opts:
                            for dp in range(4):  # pairs of dout chunks
                                po = pp.tile([128, 1024], F32, tag="smm")
                                ot = work.tile([128, 1024], F32, tag="ot", name=R + "ot")
                                for u in range(2):
                                    dc = 2 * dp + u
                                    pou = po[:, 512 * u:512 * (u + 1)]
                                    for fc in range(4):
                                        nc.tensor.matmul(
                                            pou, wo_sb[fc][:, 128 * dc:128 * (dc + 1)],
                                            yT[fc][:, sl], start=(fc == 0), stop=(fc == 3))
                                    nc.scalar.activation(ot[:, 512 * u:512 * (u + 1)], pou,
                                                         IDENT, bias=bo_sb[:, dc:dc + 1],
                                                         scale=1.0)
                                od = bass.AP(tensor=outT_dram.tensor,
                                             offset=outT_dram.offset + 256 * dp * T + SLAB * js,
                                             ap=[[T, 256], [0, 1], [1, 512]])
                                ot2 = ot[:].rearrange("p (a b) -> p a b", a=2)
                                nc.sync.dma_start(
                                    out=outT_dram[:].rearrange("(c p) t -> p c t", p=128)[
                                        :, 2 * dp:2 * dp + 2, SLAB * js:SLAB * (js + 1)],
                                    in_=ot2)
                        elif "3" in parts:
                            for dc in range(D // 128):
                                po = pp.tile([128, 512], F32, tag="sdiag")
                                for fc in range(4):
                                    nc.tensor.matmul(po[:], wo_sb[fc][:, 128 * dc:128 * (dc + 1)],
                                                     yT[fc][:, sl], start=(fc == 0), stop=(fc == 3))
                                ot = work.tile([128, 512], F32, tag="ot", name=R + "ot")
                                nc.scalar.activation(ot[:], po[:], IDENT,
                                                     bias=bo_sb[:, dc:dc + 1], scale=1.0)
                                nc.sync.dma_start(out=outT_dram[128 * dc:128 * (dc + 1), sl],
                                                  in_=ot[:])

                    if debug and rep == repeat - 1:
                        for fc in range(4):
                            nc.sync.dma_start(out=yTd[128 * fc:128 * (fc + 1), :],
                                              in_=yT[fc][:].bitcast(F32))
                        nc.sync.dma_start(out=outTd[:], in_=outT_dram[:])

            if "3" not in parts or "2" not in parts or not gN:
                dummy = constp.tile([128, 512], F32, tag="dummy")
                nc.vector.memset(dummy[:], 0.0)
                for dc in range(D // 128):
                    for js2 in range(NS):
                        nc.sync.dma_start(
                            out=outT_dram[128 * dc:128 * (dc + 1),
                                          SLAB * js2:SLAB * (js2 + 1)],
                            in_=dummy[:])

            # ================= pair ReduceScatter =================
            nc.gpsimd.collective_compute(
                "ReduceScatter", mybir.AluOpType.add,
                ins=[outT_dram[:]], outs=[rs_out[:]],
                replica_groups=[[0, 1], [2, 3], [4, 5], [6, 7]],
            )
            nc.sync.dma_start(out=out_shard[:], in_=rs_out[:])

    nc.compile()
    return nc


def get_nc(debug=False, repeat=1, parts="123", use_f32r=True, opts="MYP"):
    key = ("nc", debug, repeat, parts, use_f32r, opts)
    if key not in _CACHE:
        _CACHE[key] = _build_nc(debug, repeat, parts, use_f32r, opts)
    return _CACHE[key]


def prep_in_maps(x, mask, Wq, bq, Wk, bk, Wv, bv, Wo, bo):
    x = np.asarray(x, np.float32)
    Wq, Wk, Wv, Wo = (np.asarray(w, np.float32) for w in (Wq, Wk, Wv, Wo))
    bq, bk, bv, bo = (np.asarray(b, np.float32) for b in (bq, bk, bv, bo))
    tri = np.where(np.arange(128)[:, None] <= np.arange(128)[None, :],
                   np.float32(0), np.float32(NEG)).astype(np.float32)
    in_maps = []
    for c in range(N_CORES):
        b, g = c // 2, c % 2
        fs = slice(FL * g, FL * (g + 1))
        xT = np.ascontiguousarray(
            np.concatenate([x[b].T, np.ones((1, T), np.float32)], axis=0))
        wq_ = np.concatenate([Wq.T[:, fs], bq[None, fs]], axis=0)
        wk_ = np.concatenate([Wk.T[:, fs], bk[None, fs]], axis=0)
        wv_ = np.concatenate([Wv.T[:, fs], bv[None, fs]], axis=0)
        wo_ = np.ascontiguousarray(Wo.T[fs, :])
        bo2 = np.ascontiguousarray((bo / 2).reshape(D // 128, 128).T)
        bqc = np.ascontiguousarray(bq[fs].reshape(FL // 128, 128).T)
        bkc = np.ascontiguousarray(bk[fs].reshape(FL // 128, 128).T)
        in_maps.append({
            "xT": xT, "wq": np.ascontiguousarray(wq_),
            "wk": np.ascontiguousarray(wk_), "wv": np.ascontiguousarray(wv_),
            "wo": wo_, "bo2": bo2, "trimask": tri, "bqc": bqc, "bkc": bkc,
        })
    return in_maps


def assemble(results):
    out = np.empty((B, T, D), np.float32)
    for b in range(B):
        top = results[2 * b]["out_shard"]       # rows 0:512 of outT partial sum
        bot = results[2 * b + 1]["out_shard"]   # rows 512:1024
        out[b] = np.concatenate([top, bot], axis=0).T
    return out


def kernel(x, mask, Wq, bq, Wk, bk, Wv, bv, Wo, bo):
    from concourse.bass_utils import run_bass_kernel_spmd
    nc = get_nc()
    in_maps = prep_in_maps(x, mask, Wq, bq, Wk, bk, Wv, bv, Wo, bo)
    res = run_bass_kernel_spmd(nc, in_maps, core_ids=list(range(N_CORES)))
    return assemble(res.results)



# revision 9
# speedup vs baseline: 217.9942x; 217.9942x over previous
"""Causal self-attention Trainium2 kernel (8 NeuronCores).

Sharding: data-parallel over batch (4) x tensor-parallel over heads (2).
Core c handles batch b = c//2 and head group g = c%2 (8 of 16 heads,
feature slice [512*g, 512*(g+1))).

Per-core algorithm (T=2048, D=1024, local F=512, DK=64), f32r matmuls:
  qT/kT [512, 2048] = W.T @ x.T   (feature-major; bias via fused DVE add)
  v     [2048, 512] = x @ Wv.T    (token-major, interleaved [h, 64+1] with
                                   a ones column for the softmax denominator)
  per head h, per query slab (512 cols), tk tiles of 128 in groups of 4:
    scoresT [128, 4*512] = kT_tiles.T @ qT_cols  in one PSUM tile
    probT = exp(scoresT/8) in ONE activation     (no max subtraction)
    yT[65, 512] += [v | 1].T @ probT             (row 64 = denominator)
    yT_norm = yT[0:64] * bcast(1/denom)
  outT_partial [1024, 2048] = Wo.T @ yT (+ bo/2)
  ReduceScatter(add) over the core pair -> out shard [512, 2048]

The runtime charges a large flat cost per compute instruction, so the
design minimizes instruction count: batched activations over [128, 2048]
PSUM groups, multi-tile strided DVE copies, biases fused into existing
copy instructions (they are zero in practice).
"""
import sys, os
from contextlib import ExitStack

for _p in ("/opt/trn_rl_repo", "/root/.axon_site/_ro/trn_rl_repo"):
    if os.path.isdir(_p) and _p not in sys.path:
        sys.path.insert(0, _p)

import numpy as np

B, T, D, H = 4, 2048, 1024, 16
DK = D // H          # 64
N_CORES = 8
FL = D // 2          # 512 local features (8 heads)
HL = H // 2          # 8 local heads
SLAB = 512           # tq slab
NT = T // 128        # 16 token tiles
NS = T // SLAB       # 4 slabs
KC = D // 128        # 8 contraction chunks
VW = DK + 1          # 65: v columns per head incl. ones column
NEG = -1.0e10

_CACHE = {}


def _build_nc(debug=False, repeat=1, parts="123"):
    import concourse.bass as bass
    import concourse.tile as tile
    from concourse import bacc, mybir

    F32 = mybir.dt.float32
    F32R = mybir.dt.float32r
    EXP = mybir.ActivationFunctionType.Exp
    ADD = mybir.AluOpType.add
    MULT = mybir.AluOpType.mult

    nc = bacc.Bacc("TRN2", target_bir_lowering=False, debug=False,
                   num_devices=N_CORES)

    xT = nc.dram_tensor("xT", [D, T], F32R, kind="ExternalInput").ap()
    wq = nc.dram_tensor("wq", [D, FL], F32R, kind="ExternalInput").ap()
    wk = nc.dram_tensor("wk", [D, FL], F32R, kind="ExternalInput").ap()
    wv = nc.dram_tensor("wv", [D, FL], F32R, kind="ExternalInput").ap()
    wo = nc.dram_tensor("wo", [FL, D], F32R, kind="ExternalInput").ap()
    bqc = nc.dram_tensor("bqc", [128, FL // 128], F32, kind="ExternalInput").ap()
    bkc = nc.dram_tensor("bkc", [128, FL // 128], F32, kind="ExternalInput").ap()
    bvrep = nc.dram_tensor("bvrep", [128, FL], F32, kind="ExternalInput").ap()
    bo2 = nc.dram_tensor("bo2", [128, D // 128], F32, kind="ExternalInput").ap()
    trimask = nc.dram_tensor("trimask", [128, 128], F32, kind="ExternalInput").ap()
    out_shard = nc.dram_tensor("out_shard", [FL, T], F32, kind="ExternalOutput").ap()
    if debug:
        qTd = nc.dram_tensor("qTd", [FL, T], F32, kind="ExternalOutput").ap()
        kTd = nc.dram_tensor("kTd", [FL, T], F32, kind="ExternalOutput").ap()
        vd = nc.dram_tensor("vd", [NT * 128, HL * VW], F32, kind="ExternalOutput").ap()
        yTd = nc.dram_tensor("yTd", [FL, T], F32, kind="ExternalOutput").ap()
        outTd = nc.dram_tensor("outTd", [D, T], F32, kind="ExternalOutput").ap()

    with tile.TileContext(nc) as tc:
        with tc.tile_pool(name="const", bufs=1) as constp, \
             tc.tile_pool(name="psum", bufs=1, space="PSUM") as pp, \
             tc.tile_pool(name="psacc", bufs=2, space="PSUM") as pa, \
             tc.tile_pool(name="dram", bufs=1, space="DRAM") as dram:

            # ---- constants ----
            m_sb = constp.tile([128, 128], F32, tag="m")
            nc.sync.dma_start(out=m_sb[:], in_=trimask[:])
            bq_sb = constp.tile([128, FL // 128], F32, tag="bqc")
            nc.sync.dma_start(out=bq_sb[:], in_=bqc[:])
            bk_sb = constp.tile([128, FL // 128], F32, tag="bkc")
            nc.sync.dma_start(out=bk_sb[:], in_=bkc[:])
            bv_sb = constp.tile([128, FL], F32, tag="bvrep")
            nc.sync.dma_start(out=bv_sb[:], in_=bvrep[:])
            bo_sb = constp.tile([128, D // 128], F32, tag="bo")
            nc.sync.dma_start(out=bo_sb[:], in_=bo2[:])
            ones_f = constp.tile([1, 64], F32, tag="onesf")
            ones64 = constp.tile([1, 64], F32R, tag="ones")
            nc.vector.memset(ones_f[:], 1.0)
            nc.vector.tensor_copy(ones64[:], ones_f[:])
            ones128 = constp.tile([128, 128], F32, tag="ones128")
            nc.vector.memset(ones128[:], 1.0)

            outT_dram = dram.tile([D, T], F32)
            rs_out = dram.tile([FL, T], F32)

            for rep in range(repeat):
                R = f"r{rep}_"
                with ExitStack() as st:
                    persist = st.enter_context(tc.tile_pool(name=R + "persist", bufs=1))
                    qTbig = persist.tile([128, 4 * T], F32R, tag="qTbig", name=R + "qTbig")
                    kTbig = persist.tile([128, 4 * T], F32R, tag="kTbig", name=R + "kTbig")
                    vbig = persist.tile([128, NT * VW * HL], F32R, tag="vbig",
                                        name=R + "vbig")
                    # ones columns: col VW*HL*jt + VW*h + DK for all jt, h
                    vb_ones = bass.AP(
                        tensor=vbig.tensor, offset=vbig.offset + DK,
                        ap=[list(vbig.ap[0]), [VW * HL, NT], [VW, HL]])
                    ones_src = bass.AP(
                        tensor=ones128.tensor, offset=ones128.offset,
                        ap=[list(ones128.ap[0]), [HL, NT], [1, HL]])
                    nc.vector.tensor_copy(vb_ones, ones_src)

                    g1 = "1" in parts
                    # ================= phase 1: projections =================
                    with tc.tile_pool(name=R + "ph1", bufs=1) as ph1:
                        wq_sb = [ph1.tile([128, FL], F32R, tag=f"wq{kc}",
                                          name=R + f"wq_sb{kc}") for kc in range(KC)]
                        wk_sb = [ph1.tile([128, FL], F32R, tag=f"wk{kc}",
                                          name=R + f"wk_sb{kc}") for kc in range(KC)]
                        wv_sb = [ph1.tile([128, FL], F32R, tag=f"wv{kc}",
                                          name=R + f"wv_sb{kc}") for kc in range(KC)]
                        for kc in range(KC if g1 else 0):
                            nc.sync.dma_start(out=wq_sb[kc][:], in_=wq[128 * kc:128 * (kc + 1), :])
                            nc.sync.dma_start(out=wk_sb[kc][:], in_=wk[128 * kc:128 * (kc + 1), :])
                            nc.sync.dma_start(out=wv_sb[kc][:], in_=wv[128 * kc:128 * (kc + 1), :])

                        with tc.tile_pool(name=R + "xa", bufs=2) as xap:
                            for js in range(NS if g1 else 0):
                                sl = slice(SLAB * js, SLAB * (js + 1))
                                xa = [xap.tile([128, SLAB], F32R, tag=f"xa{kc}",
                                               name=R + f"xa{js}_{kc}") for kc in range(KC)]
                                for kc in range(KC):
                                    nc.sync.dma_start(out=xa[kc][:],
                                                      in_=xT[128 * kc:128 * (kc + 1), sl])

                                # q and k: feature-major [f, t], all 4 fc chunks
                                # in one [128, 2048] PSUM tile, one fused
                                # copy+bias DVE out
                                for wsb, bias, dstbig in ((wq_sb, bq_sb, qTbig),
                                                          (wk_sb, bk_sb, kTbig)):
                                    ps = pp.tile([128, 2048], F32, tag="smm")
                                    for fc in range(4):
                                        po = ps[:, 512 * fc:512 * (fc + 1)]
                                        fsl = slice(128 * fc, 128 * (fc + 1))
                                        for kc in range(KC):
                                            nc.tensor.matmul(po, wsb[kc][:, fsl], xa[kc][:],
                                                             start=(kc == 0), stop=(kc == KC - 1))
                                    dd = bass.AP(
                                        tensor=dstbig.tensor,
                                        offset=dstbig.offset + SLAB * js,
                                        ap=[list(dstbig.ap[0]), [T, 4], [1, SLAB]])
                                    brep = bass.AP(
                                        tensor=bias.tensor, offset=bias.offset,
                                        ap=[list(bias.ap[0]), [1, 4], [0, SLAB]])
                                    nc.vector.tensor_tensor(
                                        out=dd, in0=ps[:].rearrange("p (a b) -> p a b", a=4),
                                        in1=brep, op=ADD)

                                # v: token-major, 4 token tiles in one PSUM tile,
                                # one fused strided copy+bias out
                                ps = pp.tile([128, 2048], F32, tag="smm")
                                for tt in range(4):
                                    po = ps[:, 512 * tt:512 * (tt + 1)]
                                    tsl = slice(128 * tt, 128 * (tt + 1))
                                    for kc in range(KC):
                                        nc.tensor.matmul(po, xa[kc][:, tsl], wv_sb[kc][:],
                                                         start=(kc == 0), stop=(kc == KC - 1))
                                dst3 = bass.AP(
                                    tensor=vbig.tensor,
                                    offset=vbig.offset + VW * HL * 4 * js,
                                    ap=[list(vbig.ap[0]), [VW * HL, 4], [VW, HL], [1, DK]])
                                src3 = ps[:].rearrange("p (a h d) -> p a h d", a=4, h=HL)
                                bv3 = bass.AP(
                                    tensor=bv_sb.tensor, offset=bv_sb.offset,
                                    ap=[list(bv_sb.ap[0]), [0, 4], [DK, HL], [1, DK]])
                                nc.vector.tensor_tensor(out=dst3, in0=src3, in1=bv3, op=ADD)

                    if debug and rep == repeat - 1:
                        for fc in range(4):
                            nc.sync.dma_start(out=qTd[128 * fc:128 * (fc + 1), :],
                                              in_=qTbig[:, T * fc:T * (fc + 1)].bitcast(F32))
                            nc.sync.dma_start(out=kTd[128 * fc:128 * (fc + 1), :],
                                              in_=kTbig[:, T * fc:T * (fc + 1)].bitcast(F32))
                        for j in range(NT):
                            nc.sync.dma_start(
                                out=vd[128 * j:128 * (j + 1), :],
                                in_=vbig[:, VW * HL * j:VW * HL * (j + 1)].bitcast(F32))

                    # ============ phase 2: attention ============
                    p2 = st.enter_context(tc.tile_pool(name=R + "p2", bufs=1))
                    work = st.enter_context(tc.tile_pool(name=R + "work", bufs=2))
                    yTbig = p2.tile([128, 4 * T], F32R, tag="yTbig", name=R + "yTbig")
                    wo_sb = [p2.tile([128, D], F32R, tag=f"wo{fc}", name=R + f"wo_sb{fc}")
                             for fc in range(4)]
                    for fc in range(4):
                        nc.sync.dma_start(out=wo_sb[fc][:], in_=wo[128 * fc:128 * (fc + 1), :])

                    DIAG_OFF = (0, 512, 896, 1152)  # packed diag widths 512/384/256/128
                    for js in range(NS if "2" in parts else 0):
                        for h in range(HL):
                            hp, off = h // 2, 64 * (h % 2)
                            qh = qTbig[off:off + 64, T * hp + SLAB * js:
                                       T * hp + SLAB * (js + 1)]

                            def kt(j):
                                return kTbig[off:off + 64,
                                             T * hp + 128 * j:T * hp + 128 * (j + 1)]

                            yp = pa.tile([65, 512], F32, tag="yacc")
                            av_start = True
                            # full (unmasked) tk tiles, groups of 2
                            for g in range(2 * js):
                                ps = pp.tile([128, 2048], F32, tag="smm")
                                for m in range(2):
                                    j = 2 * g + m
                                    nc.tensor.matmul(ps[:, 512 * m:512 * (m + 1)],
                                                     kt(j), qh, start=True, stop=True)
                                pr = work.tile([128, 2048], F32R, tag="prob")
                                nc.scalar.activation(pr[:, 0:1024], ps[:, 0:1024],
                                                     EXP, scale=0.125)
                                for m in range(2):
                                    j = 2 * g + m
                                    nc.tensor.matmul(yp[:],
                                                     vbig[:, VW * HL * j + VW * h:
                                                          VW * HL * j + VW * (h + 1)],
                                                     pr[:, 512 * m:512 * (m + 1)],
                                                     start=av_start, stop=False,
                                                     skip_group_check=True)
                                    av_start = False
                            # diagonal region: tiles 4js+jl, widths 512-128jl,
                            # packed at DIAG_OFF
                            pd = pp.tile([128, 2048], F32, tag="smm")
                            for jl in range(4):
                                j, w = 4 * js + jl, 512 - 128 * jl
                                nc.tensor.matmul(pd[:, DIAG_OFF[jl]:DIAG_OFF[jl] + w],
                                                 kt(j), qh[:, 128 * jl:512],
                                                 start=True, stop=True)
                            # triangular masks at cols DIAG_OFF[jl], pairs
                            for base, stride in ((0, 512), (896, 256)):
                                pdm = bass.AP(tensor=pd.tensor, offset=pd.offset + base,
                                              ap=[list(pd.ap[0]), [stride, 2], [1, 128]])
                                mr2 = bass.AP(tensor=m_sb.tensor, offset=m_sb.offset,
                                              ap=[list(m_sb.ap[0]), [0, 2], [1, 128]])
                                nc.vector.tensor_tensor(out=pdm, in0=pdm, in1=mr2, op=ADD)
                            prd = work.tile([128, 2048], F32R, tag="prob")
                            nc.scalar.activation(prd[:, 0:1280], pd[:, 0:1280],
                                                 EXP, scale=0.125)
                            for jl in range(4):
                                j, w = 4 * js + jl, 512 - 128 * jl
                                nc.tensor.matmul(yp[:, 128 * jl:512],
                                                 vbig[:, VW * HL * j + VW * h:
                                                      VW * HL * j + VW * (h + 1)],
                                                 prd[:, DIAG_OFF[jl]:DIAG_OFF[jl] + w],
                                                 start=av_start, stop=(jl == 3),
                                                 skip_group_check=True)
                                av_start = False
                            # normalize: yT = yp[0:64] * bcast(1 / yp[64])
                            rec = work.tile([1, 512], F32R, tag="rec", name=R + "rec")
                            with nc.allow_low_precision(reason="f32r is rounded fp32"):
                                nc.vector.reciprocal(rec[:], yp[64:65, :])
                            pb = pa.tile([64, 512], F32, tag="pb")
                            nc.tensor.matmul(pb[:], ones64[:], rec[:],
                                             start=True, stop=True)
                            yun = work.tile([64, 512], F32, tag="yun", name=R + "yun")
                            nc.vector.tensor_copy(yun[:], yp[0:64, :])
                            nc.vector.tensor_tensor(
                                out=yTbig[off:off + 64,
                                          T * hp + SLAB * js:T * hp + SLAB * (js + 1)],
                                in0=yun[:], in1=pb[:], op=MULT)

                    if debug and rep == repeat - 1:
                        for fc in range(4):
                            nc.sync.dma_start(out=yTd[128 * fc:128 * (fc + 1), :],
                                              in_=yTbig[:, T * fc:T * (fc + 1)].bitcast(F32))

                    # ============ phase 3: out-projection ============
                    for js in range(NS if "3" in parts else 0):
                        sl = slice(SLAB * js, SLAB * (js + 1))
                        for dp in range(2):  # 4 dout chunks per PSUM tile
                            po = pp.tile([128, 2048], F32, tag="smm")
                            for u in range(4):
                                dc = 4 * dp + u
                                pou = po[:, 512 * u:512 * (u + 1)]
                                for fc in range(4):
                                    nc.tensor.matmul(
                                        pou, wo_sb[fc][:, 128 * dc:128 * (dc + 1)],
                                        yTbig[:, T * fc + SLAB * js:T * fc + SLAB * (js + 1)],
                                        start=(fc == 0), stop=(fc == 3))
                            ot = work.tile([128, 2048], F32, tag="ot", name=R + "ot")
                            brep = bass.AP(
                                tensor=bo_sb.tensor, offset=bo_sb.offset + 4 * dp,
                                ap=[list(bo_sb.ap[0]), [1, 4], [0, SLAB]])
                            nc.vector.tensor_tensor(
                                out=ot[:].rearrange("p (a b) -> p a b", a=4),
                                in0=po[:].rearrange("p (a b) -> p a b", a=4),
                                in1=brep, op=ADD)
                            nc.sync.dma_start(
                                out=outT_dram[:].rearrange("(c p) t -> p c t", p=128)[
                                    :, 4 * dp:4 * dp + 4, sl],
                                in_=ot[:].rearrange("p (a b) -> p a b", a=4))

                    if debug and rep == repeat - 1:
                        nc.sync.dma_start(out=outTd[:], in_=outT_dram[:])

            if "3" not in parts or "2" not in parts:
                dummy = constp.tile([128, 512], F32, tag="dummy")
                nc.vector.memset(dummy[:], 0.0)
                for dc in range(D // 128):
                    for js2 in range(NS):
                        nc.sync.dma_start(
                            out=outT_dram[128 * dc:128 * (dc + 1),
                                          SLAB * js2:SLAB * (js2 + 1)],
                            in_=dummy[:])

            # ================= pair ReduceScatter =================
            if "C" in parts or parts == "123":
                nc.gpsimd.collective_compute(
                    "ReduceScatter", mybir.AluOpType.add,
                    ins=[outT_dram[:]], outs=[rs_out[:]],
                    replica_groups=[[0, 1], [2, 3], [4, 5], [6, 7]],
                )
                nc.sync.dma_start(out=out_shard[:], in_=rs_out[:])
            else:
                nc.sync.dma_start(out=out_shard[:], in_=outT_dram[0:FL, :])

    nc.compile()
    return nc


def get_nc(debug=False, repeat=1, parts="123"):
    key = ("nc", debug, repeat, parts)
    if key not in _CACHE:
        _CACHE[key] = _build_nc(debug, repeat, parts)
    return _CACHE[key]


def prep_in_maps(x, mask, Wq, bq, Wk, bk, Wv, bv, Wo, bo):
    x = np.asarray(x, np.float32)
    Wq, Wk, Wv, Wo = (np.asarray(w, np.float32) for w in (Wq, Wk, Wv, Wo))
    bq, bk, bv, bo = (np.asarray(b, np.float32) for b in (bq, bk, bv, bo))
    tri = np.where(np.arange(128)[:, None] <= np.arange(128)[None, :],
                   np.float32(0), np.float32(NEG)).astype(np.float32)
    in_maps = []
    for c in range(N_CORES):
        b, g = c // 2, c % 2
        fs = slice(FL * g, FL * (g + 1))
        xT = np.ascontiguousarray(x[b].T)
        in_maps.append({
            "xT": xT,
            "wq": np.ascontiguousarray(Wq.T[:, fs]),
            "wk": np.ascontiguousarray(Wk.T[:, fs]),
            "wv": np.ascontiguousarray(Wv.T[:, fs]),
            "wo": np.ascontiguousarray(Wo.T[fs, :]),
            "bqc": np.ascontiguousarray(bq[fs].reshape(FL // 128, 128).T),
            "bkc": np.ascontiguousarray(bk[fs].reshape(FL // 128, 128).T),
            "bvrep": np.ascontiguousarray(np.tile(bv[fs], (128, 1))),
            "bo2": np.ascontiguousarray((bo / 2).reshape(D // 128, 128).T),
            "trimask": tri,
        })
    return in_maps


def assemble(results):
    out = np.empty((B, T, D), np.float32)
    for b in range(B):
        top = results[2 * b]["out_shard"]       # rows 0:512 of outT partial sum
        bot = results[2 * b + 1]["out_shard"]   # rows 512:1024
        out[b] = np.concatenate([top, bot], axis=0).T
    return out


def kernel(x, mask, Wq, bq, Wk, bk, Wv, bv, Wo, bo):
    from concourse.bass_utils import run_bass_kernel_spmd
    nc = get_nc()
    in_maps = prep_in_maps(x, mask, Wq, bq, Wk, bk, Wv, bv, Wo, bo)
    res = run_bass_kernel_spmd(nc, in_maps, core_ids=list(range(N_CORES)))
    return assemble(res.results)
